# revision 1
# baseline (speedup 1.0000x reference)
"""Trainium2 Bass kernel for nn_DiagnosticRNN (LSTM B=2048,T=128,V=25,H=512
-> FC 100), 8-way batch-data-parallel across NeuronCores.

Strategy
--------
Data-parallel over batch: each of the 8 cores runs the full T=128 LSTM
recurrence on BS=256 batch rows with all weights replicated (per the
sharding hint). Everything is fused on-chip; the naive approach's
[B,T,4H] x-projection (2.1 GB of DRAM traffic) is never materialized.

Per-core per-timestep (all matmul operands bf16, fp32 PSUM accumulate):
  gates[4H, BS] = W_hh_perm @ h_{t-1} + W_ihaug_perm @ [x_t; 1]
    * 16 M-tiles (permuted gate order: m = 4j+q, q in (i,f,g,o), j the
      h-chunk), N=BS=256, PSUM bank b holds M-tile pair (2b, 2b+1).
    * x-term: thin K=26 matmuls (V=25 + a ones row folding b_ih+b_hh),
      4-way row-tiled via tile_position (x replicated at partition
      offsets 0/32/64/96); even m start=True then odd m start=False so
      each PSUM bank's has_written clear happens exactly once per step.
    * W_hh term: 64 MMs, k-outer so step t+1's k-th sweep only needs
      h-chunk k -> deep cross-step pipelining with ACT/DVE.
  ScalarE: sigmoid([i_j f_j] fused 512 cols), tanh(g_j), sigmoid(o_j),
  tanh(c_j), all PSUM->SBUF, bf16 outputs.
  VectorE/GpSimd: ig = i*g, fc = f*c (GpSimd), c = ig+fc, h = o*tanh(c),
  bf16 for DVE 2x mode.
  FC epilogue: out[100, BS] = W_fc @ h (+b_fc via ACT Identity bias);
  host transposes to [BS, 100].

Host side packs/permutes/casts the weights and pre-transposes messages
into x_rep [128, T*BS] bf16 (4 replicas of [V+1, t, b] at partition
offsets 0/32/64/96). All numerics on device; bf16 operand rounding gives
~5e-3 scale-relative absmax vs the fp32 reference.
"""

import numpy as np
import ml_dtypes

import concourse.bacc as bacc
import concourse.mybir as mybir
import concourse.tile as tile
from concourse.bass_utils import run_bass_kernel_spmd

F32 = mybir.dt.float32
BF16 = mybir.dt.bfloat16
AF = mybir.ActivationFunctionType

B, T, V = 2048, 128, 25
H = 512
NCLS = 100
CORES = 8
BS = B // CORES          # 256 batch rows per core
KT = H // 128            # 4 k-tiles (h chunks)
MT = (4 * H) // 128      # 16 m-tiles
NB = 8                   # psum banks


def _gate_perm():
    """Permutation of the 4H gate dim: m-tile m=4j+q -> gate q, h-chunk j."""
    idx = []
    for j in range(4):
        for base in (0, H, 2 * H, 3 * H):           # i, f, g, o
            idx.extend(range(base + j * 128, base + (j + 1) * 128))
    return np.array(idx)


def _pack_host(messages, W_ih, W_hh, b_ih, b_hh, W_fc, b_fc):
    perm = _gate_perm()
    W_hh_p = W_hh[perm]                              # [2048, 512]
    whh = np.zeros((128, KT * MT * 128), np.float32)
    for k in range(KT):
        for m in range(MT):
            t_ = W_hh_p[m * 128:(m + 1) * 128, k * 128:(k + 1) * 128].T
            whh[:, (k * MT + m) * 128:(k * MT + m + 1) * 128] = t_
    whh = whh.astype(ml_dtypes.bfloat16)

    W_ih_p = W_ih[perm]                              # [2048, 25]
    bias_p = (b_ih + b_hh)[perm]                     # [2048]
    wih_aug = np.zeros((26, 4 * H), np.float32)
    wih_aug[:25] = W_ih_p.T
    wih_aug[25] = bias_p
    wih = np.zeros((128, 4 * H), np.float32)
    for r in range(4):
        wih[r * 32:r * 32 + 26] = wih_aug
    wih = wih.astype(ml_dtypes.bfloat16)

    wfc = np.zeros((128, KT * NCLS), np.float32)
    W_fc_T = W_fc.T                                  # [512, 100]
    for k in range(KT):
        wfc[:, k * NCLS:(k + 1) * NCLS] = W_fc_T[k * 128:(k + 1) * 128]
    wfc = wfc.astype(ml_dtypes.bfloat16)

    bfc = b_fc.astype(np.float32).reshape(NCLS, 1)

    in_maps = []
    for c in range(CORES):
        shard = messages[c * BS:(c + 1) * BS]        # [BS, T, V]
        xT = np.ascontiguousarray(shard.transpose(2, 1, 0))  # [V, T, BS]
        x_rep = np.zeros((128, T, BS), np.float32)
        for r in range(4):
            x_rep[r * 32:r * 32 + 25] = xT
            x_rep[r * 32 + 25] = 1.0
        x_rep = x_rep.reshape(128, T * BS).astype(ml_dtypes.bfloat16)
        in_maps.append({"x_rep": x_rep, "whh": whh, "wih": wih,
                        "wfc": wfc, "bfc": bfc})
    return in_maps


def _build():
    nc = bacc.Bacc("TRN2", target_bir_lowering=False, debug=False)

    x_dram = nc.dram_tensor("x_rep", [128, T * BS], BF16,
                            kind="ExternalInput").ap()
    whh_dram = nc.dram_tensor("whh", [128, KT * MT * 128], BF16,
                              kind="ExternalInput").ap()
    wih_dram = nc.dram_tensor("wih", [128, 4 * H], BF16,
                              kind="ExternalInput").ap()
    wfc_dram = nc.dram_tensor("wfc", [128, KT * NCLS], BF16,
                              kind="ExternalInput").ap()
    bfc_dram = nc.dram_tensor("bfc", [NCLS, 1], F32,
                              kind="ExternalInput").ap()
    out_dram = nc.dram_tensor("out", [NCLS, BS], F32,
                              kind="ExternalOutput").ap()

    with tile.TileContext(nc) as tc:
        with (
            tc.tile_pool(name="const", bufs=1) as cpool,
            tc.tile_pool(name="xbuf", bufs=1) as xpool,
            tc.tile_pool(name="state", bufs=1) as spool,
            tc.tile_pool(name="psum", bufs=1, space="PSUM") as ppool,
            tc.tile_pool(name="work", bufs=3) as wpool,
        ):
            whh_sb = cpool.tile([128, KT * MT * 128], BF16)
            wih_sb = cpool.tile([128, 4 * H], BF16)
            wfc_sb = cpool.tile([128, KT * NCLS], BF16)
            bfc_sb = cpool.tile([NCLS, 1], F32)
            x_sb = xpool.tile([128, T * BS], BF16)
            h_sb = spool.tile([128, KT * BS], BF16)
            c_sb = spool.tile([128, KT * BS], BF16)

            nc.sync.dma_start(whh_sb[:], whh_dram[:])
            nc.sync.dma_start(wih_sb[:], wih_dram[:])
            nc.sync.dma_start(wfc_sb[:], wfc_dram[:])
            nc.sync.dma_start(bfc_sb[:], bfc_dram[:])
            xc = T * BS // 8
            for i in range(8):
                nc.sync.dma_start(x_sb[:, i * xc:(i + 1) * xc],
                                  x_dram[:, i * xc:(i + 1) * xc])

            nc.vector.memset(h_sb[:], 0.0)
            nc.vector.memset(c_sb[:], 0.0)

            gb = []
            for b_ in range(NB):
                t_ = ppool.tile([128, 512], F32, name=f"gbank{b_}")
                gb.append(t_)

            for t in range(T):
                xs = x_sb[:, t * BS:(t + 1) * BS]
                for phase in range(2):
                    for m in range(phase, MT, 2):
                        r = (m // 2) % 4
                        nc.tensor.matmul(
                            gb[m // 2][:, (m % 2) * BS:(m % 2 + 1) * BS],
                            wih_sb[r * 32:r * 32 + 26,
                                   m * 128:(m + 1) * 128],
                            xs[r * 32:r * 32 + 26, :],
                            start=(phase == 0), stop=False,
                            tile_position=(r * 32, 0),
                        )
                for k in range(KT):
                    for m in range(MT):
                        nc.tensor.matmul(
                            gb[m // 2][:, (m % 2) * BS:(m % 2 + 1) * BS],
                            whh_sb[:, (k * MT + m) * 128:
                                   (k * MT + m + 1) * 128],
                            h_sb[:, k * BS:(k + 1) * BS],
                            start=False,
                            stop=(k == KT - 1 and m % 2 == 1),
                        )
                for j in range(4):
                    if_t = wpool.tile([128, 512], BF16, tag="if")
                    g_t = wpool.tile([128, BS], BF16, tag="g")
                    o_t = wpool.tile([128, BS], BF16, tag="o")
                    ig_t = wpool.tile([128, BS], BF16, tag="ig")
                    fc_t = wpool.tile([128, BS], BF16, tag="fc")
                    tc_t = wpool.tile([128, BS], BF16, tag="tc")
                    nc.scalar.activation(if_t[:], gb[2 * j][:], AF.Sigmoid)
                    nc.scalar.activation(g_t[:], gb[2 * j + 1][:, 0:BS],
                                         AF.Tanh)
                    nc.scalar.activation(o_t[:], gb[2 * j + 1][:, BS:2 * BS],
                                         AF.Sigmoid)
                    cj = c_sb[:, j * BS:(j + 1) * BS]
                    nc.vector.tensor_mul(ig_t[:], if_t[:, 0:BS], g_t[:])
                    nc.gpsimd.tensor_mul(fc_t[:], if_t[:, BS:2 * BS], cj)
                    nc.vector.tensor_add(cj, ig_t[:], fc_t[:])
                    nc.scalar.activation(tc_t[:], cj, AF.Tanh)
                    nc.vector.tensor_mul(h_sb[:, j * BS:(j + 1) * BS],
                                         o_t[:], tc_t[:])

            for k in range(KT):
                nc.tensor.matmul(
                    gb[0][0:NCLS, 0:BS],
                    wfc_sb[:, k * NCLS:(k + 1) * NCLS],
                    h_sb[:, k * BS:(k + 1) * BS],
                    start=(k == 0), stop=(k == KT - 1),
                )
            out_sb = cpool.tile([NCLS, BS], F32)
            nc.scalar.activation(out_sb[:], gb[0][0:NCLS, 0:BS],
                                 AF.Identity, bias=bfc_sb[:])
            nc.sync.dma_start(out_dram[:], out_sb[:])

    nc.compile()
    return nc


_NC_CACHE = None


def kernel(messages, W_ih, W_hh, b_ih, b_hh, W_fc, b_fc):
    """Full-input entry point: shard, run on 8 NeuronCores, gather."""
    global _NC_CACHE
    messages = np.asarray(messages, np.float32)
    W_ih = np.asarray(W_ih, np.float32)
    W_hh = np.asarray(W_hh, np.float32)
    b_ih = np.asarray(b_ih, np.float32)
    b_hh = np.asarray(b_hh, np.float32)
    W_fc = np.asarray(W_fc, np.float32)
    b_fc = np.asarray(b_fc, np.float32)

    in_maps = _pack_host(messages, W_ih, W_hh, b_ih, b_hh, W_fc, b_fc)
    if _NC_CACHE is None:
        _NC_CACHE = _build()
    res = run_bass_kernel_spmd(_NC_CACHE, in_maps, list(range(CORES)))
    outs = [np.ascontiguousarray(np.asarray(res.results[c]["out"]).T)
            for c in range(CORES)]
    return np.concatenate(outs, axis=0).astype(np.float32)



# revision 9
# speedup vs baseline: 1.8566x; 1.8566x over previous
"""Trainium2 Bass kernel for nn_DiagnosticRNN (LSTM B=2048,T=128,V=25,H=512
-> FC 100), 8-way batch-data-parallel across NeuronCores.

Strategy
--------
Data-parallel over batch: each of the 8 cores runs the full T=128 LSTM
recurrence on BS=256 batch rows with all weights replicated (per the
sharding hint). Everything is fused on-chip; the naive approach's
[B,T,4H] x-projection (2.1 GB of DRAM traffic) is never materialized.

v2 layout (vs the earlier per-function-split version): gates live in a
single [128, 4096] PSUM tile as four per-chunk "pairs" of banks --
pair j (h-chunk j) occupies cols [1024j, 1024j+1024) as
[i_j | f_j | o_j | g_j], 256 cols each.  This makes the three sigmoid
gates contiguous, so ScalarE runs 3 activations per pair
(tanh(g) FD=256, sigmoid(i,f,o) FD=768, tanh(c) FD=256) = 12 ACT
instructions per step instead of 16, and each pair's PSUM drains
independently (fine-grained WAR with the next step's matmuls).

Per-core per-timestep (matmul operands bf16, fp32 PSUM accumulate):
  x-term: per pair j, 4 thin K=26 matmuls (V=25 + ones row folding
    b_ih+b_hh), one per gate, on 4 distinct 32-row tile_positions so
    they stream concurrently; start=True opens each 256-col slice.
  W_hh term: k-outer sweeps (sweep k consumes h-chunk k only), so the
    next step's early sweeps overlap the previous step's activation
    chain.  TensorE emission order interleaves the x matmuls and k0/k1
    sweeps to match the order in which pairs drain and h-chunks become
    ready.
  ScalarE per pair: tanh(g), sigmoid(ifo) [PSUM src], tanh(c) [SBUF].
  VectorE: ig = i*g, c = ig+fc, h = o*tanh(c) (bf16, 2x mode);
  GpSimd: fc = f*c.
  FC epilogue: out[100, BS] = W_fc @ h (+b_fc via ACT Identity bias);
  host transposes to [BS, 100].

Host side packs/permutes/casts the weights and pre-transposes messages
into x_rep [128, T*BS] bf16 (4 replicas of [V+1, t, b] at partition
offsets 0/32/64/96). All numerics on device; bf16 operand rounding gives
~6e-3 scale-relative absmax vs the fp32 reference.
"""

import numpy as np
import ml_dtypes

import concourse.bacc as bacc
import concourse.mybir as mybir
import concourse.tile as tile
from concourse.bass_utils import run_bass_kernel_spmd

F32 = mybir.dt.float32
BF16 = mybir.dt.bfloat16
AF = mybir.ActivationFunctionType

B, T, V = 2048, 128, 25
H = 512
NCLS = 100
CORES = 8
BS = B // CORES          # 256 batch rows per core
KT = H // 128            # 4 k-tiles (h chunks)
MT = (4 * H) // 128      # 16 m-tiles
GATE_BASE = (0, H, 3 * H, 2 * H)   # q=0:i, 1:f, 2:o, 3:g (PyTorch i,f,g,o)


def _gate_perm():
    """Permutation of the 4H gate dim: m-tile m=4j+q -> gate q, h-chunk j,
    with q ordered (i, f, o, g)."""
    idx = []
    for j in range(4):
        for base in GATE_BASE:
            idx.extend(range(base + j * 128, base + (j + 1) * 128))
    return np.array(idx)


def pack_host(messages, W_ih, W_hh, b_ih, b_hh, W_fc, b_fc, T_=T):
    messages = np.asarray(messages, np.float32)
    W_ih = np.asarray(W_ih, np.float32)
    W_hh = np.asarray(W_hh, np.float32)
    b_ih = np.asarray(b_ih, np.float32)
    b_hh = np.asarray(b_hh, np.float32)
    W_fc = np.asarray(W_fc, np.float32)
    b_fc = np.asarray(b_fc, np.float32)

    perm = _gate_perm()
    W_hh_p = W_hh[perm]                              # [2048, 512]
    whh = np.zeros((128, KT * MT * 128), np.float32)
    for k in range(KT):
        for m in range(MT):
            t_ = W_hh_p[m * 128:(m + 1) * 128, k * 128:(k + 1) * 128].T
            whh[:, (k * MT + m) * 128:(k * MT + m + 1) * 128] = t_
    whh = whh.astype(ml_dtypes.bfloat16)

    W_ih_p = W_ih[perm]                              # [2048, 25]
    bias_p = (b_ih + b_hh)[perm]                     # [2048]
    wih_aug = np.zeros((26, 4 * H), np.float32)
    wih_aug[:25] = W_ih_p.T
    wih_aug[25] = bias_p
    # two row bands (0 and 32): q in {i,f} -> band 0, q in {o,g} -> band 32,
    # so concurrent x-matmuls always target distinct PSUM banks.
    wih = np.zeros((64, 4 * H), np.float32)
    for m in range(MT):
        r = 0 if (m % 4) < 2 else 32
        wih[r:r + 26, m * 128:(m + 1) * 128] = \
            wih_aug[:, m * 128:(m + 1) * 128]
    wih = wih.astype(ml_dtypes.bfloat16)

    wfc = np.zeros((128, KT * NCLS), np.float32)
    W_fc_T = W_fc.T                                  # [512, 100]
    for k in range(KT):
        wfc[:, k * NCLS:(k + 1) * NCLS] = W_fc_T[k * 128:(k + 1) * 128]
    wfc = wfc.astype(ml_dtypes.bfloat16)

    bfc = b_fc.astype(np.float32).reshape(NCLS, 1)

    in_maps = []
    for c in range(CORES):
        shard = messages[c * BS:(c + 1) * BS, :T_]   # [BS, T, V]
        xT = np.ascontiguousarray(shard.transpose(2, 1, 0))  # [V, T, BS]
        x_rep = np.zeros((64, T_, BS), np.float32)
        for r in (0, 32):
            x_rep[r:r + 25] = xT
            x_rep[r + 25] = 1.0
        x_rep = x_rep.reshape(64, T_ * BS).astype(ml_dtypes.bfloat16)
        in_maps.append({"x_rep": x_rep, "whh": whh, "wih": wih,
                        "wfc": wfc, "bfc": bfc})
    return in_maps


def build(T_=T, reps=1):
    nc = bacc.Bacc("TRN2", target_bir_lowering=False, debug=False)

    x_dram = nc.dram_tensor("x_rep", [64, T_ * BS], BF16,
                            kind="ExternalInput").ap()
    whh_dram = nc.dram_tensor("whh", [128, KT * MT * 128], BF16,
                              kind="ExternalInput").ap()
    wih_dram = nc.dram_tensor("wih", [64, 4 * H], BF16,
                              kind="ExternalInput").ap()
    wfc_dram = nc.dram_tensor("wfc", [128, KT * NCLS], BF16,
                              kind="ExternalInput").ap()
    bfc_dram = nc.dram_tensor("bfc", [NCLS, 1], F32,
                              kind="ExternalInput").ap()
    out_dram = nc.dram_tensor("out", [NCLS, BS], F32,
                              kind="ExternalOutput").ap()

    with tile.TileContext(nc) as tc:
        with (
            tc.tile_pool(name="const", bufs=1) as cpool,
            tc.tile_pool(name="xbuf", bufs=1) as xpool,
            tc.tile_pool(name="state", bufs=1) as spool,
            tc.tile_pool(name="psum", bufs=1, space="PSUM") as ppool,
            tc.tile_pool(name="work", bufs=3) as wpool,
        ):
            whh_sb = cpool.tile([128, KT * MT * 128], BF16)
            wih_sb = cpool.tile([64, 4 * H], BF16)
            wfc_sb = cpool.tile([128, KT * NCLS], BF16)
            bfc_sb = cpool.tile([NCLS, 1], F32)
            x_sb = xpool.tile([64, T_ * BS], BF16)
            h_sb = spool.tile([128, KT * BS], BF16)
            c_sb = spool.tile([128, KT * BS], BF16)

            nc.sync.dma_start(whh_sb[:], whh_dram[:])
            nc.sync.dma_start(wih_sb[:], wih_dram[:])
            nc.sync.dma_start(wfc_sb[:], wfc_dram[:])
            nc.sync.dma_start(bfc_sb[:], bfc_dram[:])
            xc = T_ * BS // 8
            for i in range(8):
                nc.sync.dma_start(x_sb[:, i * xc:(i + 1) * xc],
                                  x_dram[:, i * xc:(i + 1) * xc])

            # gates PSUM: pair j at cols [1024j, 1024j+1024) = [i|f|o|g]_j
            gp = ppool.tile([128, 4096], F32, name="gates")

            # PSUM groups are bank-granular: exactly one start (clears the
            # bank) and one stop per bank per step. Bank 2j = [i|f],
            # bank 2j+1 = [o|g]: start on the i/o x-matmuls, stop on the
            # f/g matmuls of the last k-sweep.  Two row bands (0, 32):
            # same-band matmuls serialize on the PE so the two concurrent
            # streams (bands 0 vs 32) always write distinct PSUM banks --
            # same-bank concurrent writes crash the device.
            def xmm(xs, j):
                for q in (0, 2, 1, 3):
                    m = 4 * j + q
                    r = 0 if q < 2 else 32
                    nc.tensor.matmul(
                        gp[:, 1024 * j + 256 * q:1024 * j + 256 * (q + 1)],
                        wih_sb[r:r + 26, m * 128:(m + 1) * 128],
                        xs[r:r + 26, :],
                        start=(q % 2 == 0), stop=False,
                        tile_position=(r, 0),
                    )

            def kmm(k, j):
                for q in range(4):
                    m = 4 * j + q
                    nc.tensor.matmul(
                        gp[:, 1024 * j + 256 * q:1024 * j + 256 * (q + 1)],
                        whh_sb[:, (k * MT + m) * 128:(k * MT + m + 1) * 128],
                        h_sb[:, k * BS:(k + 1) * BS],
                        start=False,
                        stop=(k == KT - 1 and q % 2 == 1),
                    )

            def acts_pre(j):
                g_t = wpool.tile([128, BS], BF16, tag=f"g{j}")
                sfo = wpool.tile([128, 3 * BS], BF16, tag=f"sfo{j}")
                nc.scalar.activation(g_t[:], gp[:, 1024 * j + 768:
                                                1024 * j + 1024], AF.Tanh)
                nc.scalar.activation(sfo[:], gp[:, 1024 * j:
                                                1024 * j + 768], AF.Sigmoid)
                return g_t, sfo

            def chain_post(j, g_t, sfo):
                ig_t = wpool.tile([128, BS], BF16, tag=f"ig{j}")
                fc_t = wpool.tile([128, BS], BF16, tag=f"fc{j}")
                tc_t = wpool.tile([128, BS], BF16, tag=f"tc{j}")
                cj = c_sb[:, j * BS:(j + 1) * BS]
                nc.vector.tensor_mul(ig_t[:], sfo[:, 0:BS], g_t[:])
                nc.gpsimd.tensor_mul(fc_t[:], sfo[:, BS:2 * BS], cj)
                nc.vector.tensor_add(cj, ig_t[:], fc_t[:])
                nc.scalar.activation(tc_t[:], cj, AF.Tanh)
                nc.vector.tensor_mul(h_sb[:, j * BS:(j + 1) * BS],
                                     sfo[:, 2 * BS:3 * BS], tc_t[:])

            for rep in range(reps):
                nc.vector.memset(h_sb[:], 0.0)
                nc.vector.memset(c_sb[:], 0.0)

                for t in range(T_):
                    xs = x_sb[:, t * BS:(t + 1) * BS]
                    # TensorE order: interleave x-opens and early k-sweeps
                    # to match pair-drain / h-chunk readiness.
                    xmm(xs, 0)
                    xmm(xs, 1)
                    kmm(0, 0)
                    kmm(0, 1)
                    xmm(xs, 2)
                    kmm(0, 2)
                    kmm(1, 0)
                    kmm(1, 1)
                    kmm(1, 2)
                    xmm(xs, 3)
                    kmm(0, 3)
                    kmm(1, 3)
                    for j in range(4):
                        kmm(2, j)
                    for j in range(4):
                        kmm(3, j)
                    # activation chains, pair-staggered
                    g0, s0 = acts_pre(0)
                    g1, s1 = acts_pre(1)
                    chain_post(0, g0, s0)
                    g2, s2 = acts_pre(2)
                    chain_post(1, g1, s1)
                    g3, s3 = acts_pre(3)
                    chain_post(2, g2, s2)
                    chain_post(3, g3, s3)

            for k in range(KT):
                nc.tensor.matmul(
                    gp[0:NCLS, 0:BS],
                    wfc_sb[:, k * NCLS:(k + 1) * NCLS],
                    h_sb[:, k * BS:(k + 1) * BS],
                    start=(k == 0), stop=(k == KT - 1),
                )
            out_sb = cpool.tile([NCLS, BS], F32)
            nc.scalar.activation(out_sb[:], gp[0:NCLS, 0:BS],
                                 AF.Identity, bias=bfc_sb[:])
            nc.sync.dma_start(out_dram[:], out_sb[:])

    nc.compile()
    return nc


_NC_CACHE = None


def kernel(messages, W_ih, W_hh, b_ih, b_hh, W_fc, b_fc):
    """Full-input entry point: shard, run on 8 NeuronCores, gather."""
    global _NC_CACHE
    in_maps = pack_host(messages, W_ih, W_hh, b_ih, b_hh, W_fc, b_fc)
    if _NC_CACHE is None:
        _NC_CACHE = build()
    res = run_bass_kernel_spmd(_NC_CACHE, in_maps, list(range(CORES)))
    outs = [np.ascontiguousarray(np.asarray(res.results[c]["out"]).T)
            for c in range(CORES)]
    return np.concatenate(outs, axis=0).astype(np.float32)


# revision 10
# speedup vs baseline: 2.3329x; 1.2565x over previous
"""Trainium2 Bass kernel for nn_DiagnosticRNN (LSTM B=2048,T=128,V=25,H=512
-> FC 100), 8-way batch-data-parallel across NeuronCores.

Strategy
--------
Data-parallel over batch: each of the 8 cores runs the full T=128 LSTM
recurrence on BS=256 batch rows with all weights replicated (per the
sharding hint). Everything is fused on-chip; the naive approach's
[B,T,4H] x-projection (2.1 GB of DRAM traffic) is never materialized.

v2 layout (vs the earlier per-function-split version): gates live in a
single [128, 4096] PSUM tile as four per-chunk "pairs" of banks --
pair j (h-chunk j) occupies cols [1024j, 1024j+1024) as
[i_j | f_j | o_j | g_j], 256 cols each.  This makes the three sigmoid
gates contiguous, so ScalarE runs 3 activations per pair
(tanh(g) FD=256, sigmoid(i,f,o) FD=768, tanh(c) FD=256) = 12 ACT
instructions per step instead of 16, and each pair's PSUM drains
independently (fine-grained WAR with the next step's matmuls).

Per-core per-timestep (matmul operands bf16, fp32 PSUM accumulate):
  x-term: per pair j, 4 thin K=26 matmuls (V=25 + ones row folding
    b_ih+b_hh), one per gate, on 4 distinct 32-row tile_positions so
    they stream concurrently; start=True opens each 256-col slice.
  W_hh term: k-outer sweeps (sweep k consumes h-chunk k only), so the
    next step's early sweeps overlap the previous step's activation
    chain.  TensorE emission order interleaves the x matmuls and k0/k1
    sweeps to match the order in which pairs drain and h-chunks become
    ready.
  ScalarE per pair: tanh(g), sigmoid(ifo) [PSUM src], tanh(c) [SBUF].
  VectorE: ig = i*g, c = ig+fc, h = o*tanh(c) (bf16, 2x mode);
  GpSimd: fc = f*c.
  FC epilogue: out[100, BS] = W_fc @ h (+b_fc via ACT Identity bias);
  host transposes to [BS, 100].

Host side packs/permutes/casts the weights and pre-transposes messages
into x_rep [128, T*BS] bf16 (4 replicas of [V+1, t, b] at partition
offsets 0/32/64/96). All numerics on device; bf16 operand rounding gives
~6e-3 scale-relative absmax vs the fp32 reference.
"""

import numpy as np
import ml_dtypes

import concourse.bacc as bacc
import concourse.mybir as mybir
import concourse.tile as tile
from concourse.bass_utils import run_bass_kernel_spmd

F32 = mybir.dt.float32
BF16 = mybir.dt.bfloat16
AF = mybir.ActivationFunctionType

B, T, V = 2048, 128, 25
H = 512
NCLS = 100
CORES = 8
BS = B // CORES          # 256 batch rows per core
KT = H // 128            # 4 k-tiles (h chunks)
MT = (4 * H) // 128      # 16 m-tiles
GATE_BASE = (0, H, 3 * H, 2 * H)   # q=0:i, 1:f, 2:o, 3:g (PyTorch i,f,g,o)


def _gate_perm():
    """Permutation of the 4H gate dim: m-tile m=4j+q -> gate q, h-chunk j,
    with q ordered (i, f, o, g)."""
    idx = []
    for j in range(4):
        for base in GATE_BASE:
            idx.extend(range(base + j * 128, base + (j + 1) * 128))
    return np.array(idx)


def pack_host(messages, W_ih, W_hh, b_ih, b_hh, W_fc, b_fc, T_=T):
    messages = np.asarray(messages, np.float32)
    W_ih = np.asarray(W_ih, np.float32)
    W_hh = np.asarray(W_hh, np.float32)
    b_ih = np.asarray(b_ih, np.float32)
    b_hh = np.asarray(b_hh, np.float32)
    W_fc = np.asarray(W_fc, np.float32)
    b_fc = np.asarray(b_fc, np.float32)

    perm = _gate_perm()
    W_hh_p = W_hh[perm]                              # [2048, 512]
    whh = np.zeros((128, KT * MT * 128), np.float32)
    for k in range(KT):
        for m in range(MT):
            t_ = W_hh_p[m * 128:(m + 1) * 128, k * 128:(k + 1) * 128].T
            whh[:, (k * MT + m) * 128:(k * MT + m + 1) * 128] = t_
    whh = whh.astype(ml_dtypes.bfloat16)

    W_ih_p = W_ih[perm]                              # [2048, 25]
    bias_p = (b_ih + b_hh)[perm]                     # [2048]
    wih_aug = np.zeros((26, 4 * H), np.float32)
    wih_aug[:25] = W_ih_p.T
    wih_aug[25] = bias_p
    # two row bands (0 and 32): q in {i,f} -> band 0, q in {o,g} -> band 32,
    # so concurrent x-matmuls always target distinct PSUM banks.
    wih = np.zeros((64, 4 * H), np.float32)
    for m in range(MT):
        r = 0 if (m % 4) < 2 else 32
        wih[r:r + 26, m * 128:(m + 1) * 128] = \
            wih_aug[:, m * 128:(m + 1) * 128]
    wih = wih.astype(ml_dtypes.bfloat16)

    wfc = np.zeros((128, KT * NCLS), np.float32)
    W_fc_T = W_fc.T                                  # [512, 100]
    for k in range(KT):
        wfc[:, k * NCLS:(k + 1) * NCLS] = W_fc_T[k * 128:(k + 1) * 128]
    wfc = wfc.astype(ml_dtypes.bfloat16)

    bfc = b_fc.astype(np.float32).reshape(NCLS, 1)

    in_maps = []
    for c in range(CORES):
        shard = messages[c * BS:(c + 1) * BS, :T_]   # [BS, T, V]
        xT = np.ascontiguousarray(shard.transpose(2, 1, 0))  # [V, T, BS]
        x_rep = np.zeros((64, T_, BS), np.float32)
        for r in (0, 32):
            x_rep[r:r + 25] = xT
            x_rep[r + 25] = 1.0
        x_rep = x_rep.reshape(64, T_ * BS).astype(ml_dtypes.bfloat16)
        in_maps.append({"x_rep": x_rep, "whh": whh, "wih": wih,
                        "wfc": wfc, "bfc": bfc})
    return in_maps


def build(T_=T, reps=1):
    nc = bacc.Bacc("TRN2", target_bir_lowering=False, debug=False)

    x_dram = nc.dram_tensor("x_rep", [64, T_ * BS], BF16,
                            kind="ExternalInput").ap()
    whh_dram = nc.dram_tensor("whh", [128, KT * MT * 128], BF16,
                              kind="ExternalInput").ap()
    wih_dram = nc.dram_tensor("wih", [64, 4 * H], BF16,
                              kind="ExternalInput").ap()
    wfc_dram = nc.dram_tensor("wfc", [128, KT * NCLS], BF16,
                              kind="ExternalInput").ap()
    bfc_dram = nc.dram_tensor("bfc", [NCLS, 1], F32,
                              kind="ExternalInput").ap()
    out_dram = nc.dram_tensor("out", [NCLS, BS], F32,
                              kind="ExternalOutput").ap()

    with tile.TileContext(nc) as tc:
        with (
            tc.tile_pool(name="const", bufs=1) as cpool,
            tc.tile_pool(name="xbuf", bufs=1) as xpool,
            tc.tile_pool(name="state", bufs=1) as spool,
            tc.tile_pool(name="psum", bufs=1, space="PSUM") as ppool,
            tc.tile_pool(name="work", bufs=3) as wpool,
        ):
            whh_sb = cpool.tile([128, KT * MT * 128], BF16)
            wih_sb = cpool.tile([64, 4 * H], BF16)
            wfc_sb = cpool.tile([128, KT * NCLS], BF16)
            bfc_sb = cpool.tile([NCLS, 1], F32)
            x_sb = xpool.tile([64, T_ * BS], BF16)
            h_sb = spool.tile([128, KT * BS], BF16)
            c_sb = spool.tile([128, KT * BS], BF16)

            nc.sync.dma_start(whh_sb[:], whh_dram[:])
            nc.sync.dma_start(wih_sb[:], wih_dram[:])
            nc.sync.dma_start(wfc_sb[:], wfc_dram[:])
            nc.sync.dma_start(bfc_sb[:], bfc_dram[:])
            xc = T_ * BS // 8
            for i in range(8):
                nc.sync.dma_start(x_sb[:, i * xc:(i + 1) * xc],
                                  x_dram[:, i * xc:(i + 1) * xc])

            # gates PSUM: one tile per chunk j (banks 2j, 2j+1), layout
            # [i|f|o|g], 256 cols each. Chunks {0,1} and {2,3} form two
            # half-steps that ping-pong: while TensorE fills one half's
            # banks, ScalarE drains the other's, so the PSUM
            # write-after-read never stalls the PE.
            gp = []
            for j in range(4):
                t_ = ppool.tile([128, 1024], F32, name=f"gpair{j}")
                gp.append(t_)

            # PSUM groups are bank-granular: exactly one start (clears the
            # bank) and one stop per bank per step. Bank 2j = [i|f],
            # bank 2j+1 = [o|g]: start on the i/o x-matmuls, stop on the
            # f/g matmuls of the last k-sweep.  Two row bands (0, 32):
            # same-band matmuls serialize on the PE so the two concurrent
            # streams (bands 0 vs 32) always write distinct PSUM banks --
            # same-bank concurrent writes crash the device.
            def xmm(xs, j):
                for q in (0, 2, 1, 3):
                    m = 4 * j + q
                    r = 0 if q < 2 else 32
                    nc.tensor.matmul(
                        gp[j][:, 256 * q:256 * (q + 1)],
                        wih_sb[r:r + 26, m * 128:(m + 1) * 128],
                        xs[r:r + 26, :],
                        start=(q % 2 == 0), stop=False,
                        tile_position=(r, 0),
                    )

            def kmm(k, j):
                for q in range(4):
                    m = 4 * j + q
                    nc.tensor.matmul(
                        gp[j][:, 256 * q:256 * (q + 1)],
                        whh_sb[:, (k * MT + m) * 128:(k * MT + m + 1) * 128],
                        h_sb[:, k * BS:(k + 1) * BS],
                        start=False,
                        stop=(k == KT - 1 and q % 2 == 1),
                    )

            def acts_pre(j):
                g_t = wpool.tile([128, BS], BF16, tag=f"g{j}")
                sfo = wpool.tile([128, 3 * BS], BF16, tag=f"sfo{j}")
                nc.scalar.activation(g_t[:], gp[j][:, 768:1024], AF.Tanh)
                nc.scalar.activation(sfo[:], gp[j][:, 0:768], AF.Sigmoid)
                return g_t, sfo

            def chain_post(j, g_t, sfo):
                ig_t = wpool.tile([128, BS], BF16, tag=f"ig{j}")
                fc_t = wpool.tile([128, BS], BF16, tag=f"fc{j}")
                tc_t = wpool.tile([128, BS], BF16, tag=f"tc{j}")
                cj = c_sb[:, j * BS:(j + 1) * BS]
                nc.vector.tensor_mul(ig_t[:], sfo[:, 0:BS], g_t[:])
                nc.vector.tensor_mul(fc_t[:], sfo[:, BS:2 * BS], cj)
                nc.vector.tensor_add(cj, ig_t[:], fc_t[:])
                nc.scalar.activation(tc_t[:], cj, AF.Tanh)
                nc.vector.tensor_mul(h_sb[:, j * BS:(j + 1) * BS],
                                     sfo[:, 2 * BS:3 * BS], tc_t[:])

            for rep in range(reps):
                nc.vector.memset(h_sb[:], 0.0)
                nc.vector.memset(c_sb[:], 0.0)

                for t in range(T_):
                    xs = x_sb[:, t * BS:(t + 1) * BS]
                    # half L: chunks {0,1}
                    xmm(xs, 0)
                    xmm(xs, 1)
                    for k in range(KT):
                        kmm(k, 0)
                        kmm(k, 1)
                    # half H: chunks {2,3} on the other PSUM banks; its
                    # matmuls cover half L's activation chains.
                    xmm(xs, 2)
                    xmm(xs, 3)
                    for k in range(KT):
                        kmm(k, 2)
                        kmm(k, 3)
                    g0, s0 = acts_pre(0)
                    g1, s1 = acts_pre(1)
                    chain_post(0, g0, s0)
                    chain_post(1, g1, s1)
                    g2, s2 = acts_pre(2)
                    g3, s3 = acts_pre(3)
                    chain_post(2, g2, s2)
                    chain_post(3, g3, s3)

            for k in range(KT):
                nc.tensor.matmul(
                    gp[0][0:NCLS, 0:BS],
                    wfc_sb[:, k * NCLS:(k + 1) * NCLS],
                    h_sb[:, k * BS:(k + 1) * BS],
                    start=(k == 0), stop=(k == KT - 1),
                )
            out_sb = cpool.tile([NCLS, BS], F32)
            nc.scalar.activation(out_sb[:], gp[0][0:NCLS, 0:BS],
                                 AF.Identity, bias=bfc_sb[:])
            nc.sync.dma_start(out_dram[:], out_sb[:])

    nc.compile()
    return nc


_NC_CACHE = None


def kernel(messages, W_ih, W_hh, b_ih, b_hh, W_fc, b_fc):
    """Full-input entry point: shard, run on 8 NeuronCores, gather."""
    global _NC_CACHE
    in_maps = pack_host(messages, W_ih, W_hh, b_ih, b_hh, W_fc, b_fc)
    if _NC_CACHE is None:
        _NC_CACHE = build()
    res = run_bass_kernel_spmd(_NC_CACHE, in_maps, list(range(CORES)))
    outs = [np.ascontiguousarray(np.asarray(res.results[c]["out"]).T)
            for c in range(CORES)]
    return np.concatenate(outs, axis=0).astype(np.float32)
